# revision 27
# baseline (speedup 1.0000x reference)
"""BitLinear (activation int8-quant + ternary weight) Trainium2 kernel, v5.

Strategy (8 NeuronCores, token-parallel):
  - x [2,8192,2048] -> flat [16384, 2048]; core c gets a contiguous slice of
    2048 tokens.
  - Weight is pre-quantized on host (standard inference practice): w_scale =
    mean(|W|) via jax-CPU (bit-identical to reference), ternary w_q in
    {-1,0,+1} as bf16 (exact), transposed to [P, NK, D_OUT] k-tile layout.
  - Per-core activation path (all on device):
      abs-max per token -> qf = 127/s (dual-scalar op + reciprocal)
      ACT1: xq16 = f16(x*qf + 1536)  -- f16 ulp in [1024,2048) is exactly 1,
            so the downcast rounds RNE to integer.
      PE transpose (is_transpose matmul, 1 cyc/row) of each 128x128 f16 tile
            into PSUM -- no DMA-xbar transposes, which serialize against all
            in-flight DMA traffic.
      ACT2: xqT = bf16(psumT - 1536)  -- the un-magic pass doubles as the
            PSUM->SBUF evacuation, so the transpose costs only ~5% PE time.
  - PE: bf16 matmuls, stationary = xqT k-tile [128,128], moving = wq k-tile
    [128,512]; exact integer accumulation in fp32 PSUM (|acc| <= 2^18).
    GEMM runs in two 2-bank halves (n in {0,1}, then {2,3}) so 4 PSUM banks
    double-buffer the GEMM while 4 banks serve the transposes.
  - Post: ACT relu(acc*gf) -> A f32, DVE square -> bf16 out (0.2% norm err,
    gate is 2e-2).
  - Emission is software-pipelined: transposes for block m+1 are issued on
    the PE stream between GEMM m-1 and GEMM m, so ACT2(m+1) overlaps GEMM m.
"""

import sys

if "/opt/trn_rl_repo" not in sys.path:
    sys.path.insert(0, "/opt/trn_rl_repo")

import numpy as np

N_CORES = 8
P = 128
TOK_TOTAL = 16384
TOK = TOK_TOTAL // N_CORES  # 2048 tokens per core
D_IN = 2048
D_OUT = 2048
NK = D_IN // P  # 16 contraction tiles
NM = TOK // P  # 16 token blocks per core
NCHUNK = 512  # psum bank free dim (f32)
NN = D_OUT // NCHUNK  # 4
# f16 round-to-integer magic: adding 1536 puts v in [1408.5, 1663.5] inside
# [1024, 2048) where the f16 ulp is exactly 1, so the f32->f16 downcast
# rounds RNE to an integer.
MAGIC16 = 1536.0

_tile_patched = False


def _patch_tile_drain():
    """walrus in this container rejects >2 sem waits on the TileContext exit
    Drain ("Too many sync wait commands").  Split the excess waits onto
    explicit SP wait_ge instructions (same semantics: all waits complete
    before the semaphore free + final barrier)."""
    global _tile_patched
    if _tile_patched:
        return
    import concourse.tile as tile
    from bass_rust import ScopedClock

    def patched(self, tick_clock, wait_clock):
        nc_ = self.nc
        drain_inst = nc_.sync.drain()
        wait_clock.add_sem_waits(
            drain_inst.ins, ScopedClock({None: tick_clock.global_clock})
        )
        waits = list(drain_inst.ins.sync_info.on_wait or [])
        if len(waits) > 1:
            drain_inst.ins.sync_info.on_wait = waits[:1]
            name_to_sem = {}
            for key, h in self.sems.allocated().items():
                name_to_sem[getattr(h, "name", str(key))] = h
            for w in waits[1:]:
                nc_.sync.wait_ge(name_to_sem[w.ant_name], w.wait_value)
        nc_.all_engine_barrier()
        popped = nc_._tile_sem_poison_stack.pop()
        assert popped is self._sem_poison
        nc_.clear_and_free_semaphores(list(self.sems.allocated().values()))
        nc_.all_engine_barrier()

    tile.TileContext._drain_and_barrier = patched
    _tile_patched = True


def _split_excess_waits(nc, max_waits: int = 1):
    """walrus's setupSyncWait caps the number of semaphore waits a single
    instruction can carry.  Tile's scheduler freely attaches more.  Move the
    excess onto wait-only EventSemaphore carrier instructions inserted just
    before the over-subscribed instruction on the same engine (program order
    on one engine => identical semantics)."""
    from concourse import mybir

    n_split = 0
    for fn in nc.m.functions:
        for bb in fn.blocks:
            insts = bb.instructions
            i = 0
            while i < len(insts):
                inst = insts[i]
                si = getattr(inst, "sync_info", None)
                waits = list(si.on_wait) if (si is not None and si.on_wait) else []
                limit = 0 if type(inst).__name__ == "InstDmaTransposeAnt" else max_waits
                if len(waits) <= limit:
                    i += 1
                    continue
                keep = waits[-limit:] if limit else []
                extras = waits[: len(waits) - limit]
                pos = i
                for j in range(0, len(extras), max_waits):
                    ev = mybir.InstEventSemaphore(
                        name=f"wsplit_{inst.name}_{j}_{n_split}",
                        engine=inst.engine,
                        ins=[],
                        outs=[],
                        sync_info=mybir.SyncInfo(
                            on_wait=extras[j : j + max_waits], on_update=[]
                        ),
                    )
                    try:
                        nc.register_instruction(ev, overwrite=True)
                    except Exception:
                        pass
                    insts.insert(pos, ev)
                    pos += 1
                inst.sync_info.on_wait = keep
                n_split += 1
                i = pos + 1
    return n_split


def build_program(w_scale: float):
    """Build the per-core Bass program (same program runs SPMD on all 8
    cores; per-core data arrives via the input map)."""
    import concourse.bass as bass
    import concourse.tile as tile
    from concourse import mybir

    f32 = mybir.dt.float32
    f16 = mybir.dt.float16
    bf16 = mybir.dt.bfloat16
    fp8 = mybir.dt.float8e4
    AF = mybir.ActivationFunctionType
    ALU = mybir.AluOpType
    AX = mybir.AxisListType

    _patch_tile_drain()

    ws127 = float(np.float32(w_scale) / np.float32(127.0))

    nc = bass.Bass("TRN2", target_bir_lowering=False, debug=False)
    xs = nc.dram_tensor("xs", [TOK, D_IN], f32, kind="ExternalInput").ap()
    wb = nc.dram_tensor("wb", [P, NK * D_OUT], fp8, kind="ExternalInput").ap()
    y = nc.dram_tensor("y", [TOK, D_OUT], bf16, kind="ExternalOutput").ap()

    with tile.TileContext(nc) as tc:
        with (
            tc.tile_pool(name="wq", bufs=1) as wq_pool,
            tc.tile_pool(name="xin", bufs=3) as x_pool,
            tc.tile_pool(name="xq16", bufs=3) as xq_pool,
            tc.tile_pool(name="xqT16", bufs=3) as xqT16_pool,
            tc.tile_pool(name="xqt", bufs=3) as xqt_pool,
            tc.tile_pool(name="scal", bufs=24) as s_pool,
            tc.tile_pool(name="gpsum", bufs=8, space="PSUM") as gpsum_pool,
            tc.tile_pool(name="outa", bufs=2) as a_pool,
            tc.tile_pool(name="outy", bufs=2) as y_pool,
            tc.tile_pool(name="consts", bufs=1) as c_pool,
        ):
            cmagic = c_pool.tile([P, 1], f32)
            nc.vector.memset(cmagic[:], MAGIC16)
            warm = c_pool.tile([P, 1], f32)
            nc.scalar.activation(warm[:], cmagic[:], AF.Identity, bias=0.0, scale=1.0)
            cneg = c_pool.tile([P, 1], f32)
            nc.vector.memset(cneg[:], -MAGIC16)

            # persistent ternary W^T bf16 [128, k*D_OUT]; per-k chunk loads
            # split across the gpsimd and vector queues so the full 8.4MB
            # lands by ~15us
            wqd = wq_pool.tile([P, NK * D_OUT], fp8)
            for k in range(0, NK, 2):
                nc.gpsimd.dma_start(
                    wqd[:, k * D_OUT : (k + 1) * D_OUT],
                    wb[:, k * D_OUT : (k + 1) * D_OUT],
                )

            state = {}

            def emit_quant(m):
                xf = x_pool.tile([P, D_IN], f32, tag="xf", name=f"xf_{m}")
                s0 = s_pool.tile([P, 1], f32, tag="s0", name=f"s0_{m}")
                if m == 0:
                    # split the first x load + absmax into halves so the
                    # critical path overlaps the DMA
                    H = D_IN // 2
                    nc.sync.dma_start(xf[:, 0:H], xs[0:P, 0:H])
                    nc.sync.dma_start(xf[:, H:], xs[0:P, H:])
                    for k in (1, 3, 5, 7):
                        nc.sync.dma_start(
                            wqd[:, k * D_OUT : (k + 1) * D_OUT],
                            wb[:, k * D_OUT : (k + 1) * D_OUT],
                        )
                    sh = s_pool.tile([P, 1], f32, tag="sh", name="sh_0")
                    nc.vector.tensor_reduce(
                        sh[:], xf[:, 0:H], AX.X, ALU.max, apply_absolute_value=True
                    )
                    s1h = s_pool.tile([P, 1], f32, tag="s1h", name="s1h_0")
                    nc.vector.tensor_reduce(
                        s1h[:], xf[:, H:], AX.X, ALU.max, apply_absolute_value=True
                    )
                    nc.vector.tensor_tensor(s0[:], sh[:], s1h[:], ALU.max)
                else:
                    nc.sync.dma_start(xf[:], xs[m * P : (m + 1) * P, :])
                    if m == 1:
                        for k in (9, 11, 13, 15):
                            nc.sync.dma_start(
                                wqd[:, k * D_OUT : (k + 1) * D_OUT],
                                wb[:, k * D_OUT : (k + 1) * D_OUT],
                            )
                    nc.vector.tensor_reduce(
                        s0[:], xf[:], AX.X, ALU.max, apply_absolute_value=True
                    )
                u = s_pool.tile([P, 1], f32, tag="u", name=f"u_{m}")
                nc.vector.tensor_scalar(
                    u[:], s0[:], 1e-5, 1.0 / 127.0, ALU.max, ALU.mult
                )
                qf = s_pool.tile([P, 1], f32, tag="qf", name=f"qf_{m}")
                nc.vector.reciprocal(qf[:], u[:])
                gf = s_pool.tile([P, 1], f32, tag="gf", name=f"gf_{m}")
                nc.vector.tensor_scalar(gf[:], qf[:], ws127, None, ALU.mult)
                xq = xq_pool.tile([P, D_IN], f16, tag="xq", name=f"xq_{m}")
                nc.scalar.activation(
                    xq[:], xf[:], AF.Identity, bias=cmagic[:, 0:1], scale=qf[:, 0:1]
                )
                state[m] = {"xq": xq, "gf": gf}

            def emit_transpose(m):
                st = state[m]
                xq = st["xq"]
                # DMA-xbar transpose (off the PE); its queue serialization
                # against in-flight DMAs is absorbed by the ~1.5-block
                # pipeline slack, and the un-magic ACT pass converts to bf16
                xqT = xqT16_pool.tile([P, NK * P], f16, tag="xqT", name=f"xqT_{m}")
                eng = nc.sync if (m % 2 == 0) else nc.scalar
                eng.dma_start_transpose(
                    xqT[:].rearrange("p (k t) -> p k t", k=NK), xq[:]
                )
                xqt = xqt_pool.tile([P, NK * P], bf16, tag="xqt", name=f"xqt_{m}")
                nc.scalar.activation(
                    xqt[:], xqT[:], AF.Identity, bias=cneg[:, 0:1], scale=1.0
                )
                st["xqt"] = xqt

            def emit_gemm(m):
                st = state[m]
                xqt = st["xqt"]
                gf = st["gf"]
                A = a_pool.tile([P, D_OUT], f32, tag="A", name=f"A_{m}")
                Y = y_pool.tile([P, D_OUT], bf16, tag="Y", name=f"Y_{m}")
                pss = []
                for n in range(NN):
                    ps = gpsum_pool.tile(
                        [P, NCHUNK], f32, tag="ps", name=f"ps_{m}_{n}"
                    )
                    pss.append((n, ps))
                for k in range(NK):
                    for n, ps in pss:
                        nc.tensor.matmul(
                            ps[:],
                            xqt[:, k * P : (k + 1) * P],
                            wqd[:, k * D_OUT + n * NCHUNK : k * D_OUT + (n + 1) * NCHUNK],
                            start=(k == 0),
                            stop=(k == NK - 1),
                        )
                for half in range(2):
                    for n, ps in pss[2 * half : 2 * half + 2]:
                        nc.scalar.activation(
                            A[:, n * NCHUNK : (n + 1) * NCHUNK],
                            ps[:],
                            AF.Relu,
                            bias=0.0,
                            scale=gf[:, 0:1],
                        )
                    hs = slice(half * 2 * NCHUNK, (half + 1) * 2 * NCHUNK)
                    nc.vector.tensor_tensor(Y[:, hs], A[:, hs], A[:, hs], ALU.mult)
                    nc.sync.dma_start(y[m * P : (m + 1) * P, hs], Y[:, hs])

            # software pipeline on the PE stream:
            #   T0, G0, T1, T2, G1, T3, G2, ... (block-1 quant isn't ready
            #   when G0 starts, so T1 must not block the queue before G0)
            emit_quant(0)
            emit_transpose(0)
            emit_quant(1)
            emit_quant(2)
            emit_gemm(0)
            emit_transpose(1)
            for m in range(3, NM):
                emit_quant(m)
                emit_transpose(m - 1)
                emit_gemm(m - 2)
            emit_transpose(NM - 1)
            emit_gemm(NM - 2)
            emit_gemm(NM - 1)

    _split_excess_waits(nc)
    return nc


def _w_scale_like_reference(weight: np.ndarray) -> float:
    """mean(|W|) computed with jax on CPU so it is bit-identical to the
    reference's jnp.mean(jnp.abs(weight))."""
    try:
        import jax
        import jax.numpy as jnp

        cpu = jax.devices("cpu")[0]
        with jax.default_device(cpu):
            return float(jnp.mean(jnp.abs(jnp.asarray(weight, dtype=jnp.float32))))
    except Exception:
        return float(np.float32(np.abs(weight).astype(np.float64).mean()))


def _prep_weight(weight: np.ndarray, w_scale: float) -> np.ndarray:
    """Host-side weight quantization (offline in a real deployment): ternary
    {-1,0,1} with threshold 0.5*w_scale (same f32 compares as the reference),
    transposed, bf16, k-tile layout wb[p, k*D_OUT + o] = W_q^T[128k+p, o]."""
    import ml_dtypes

    w = weight.astype(np.float32, copy=False)
    thr = np.float32(0.5) * np.float32(w_scale)
    wq = np.where(
        w > thr, np.float32(1.0), np.where(w < -thr, np.float32(-1.0), np.float32(0.0))
    )
    wqT = np.ascontiguousarray(wq.T)  # [in, out]
    w4 = wqT.reshape(NK, P, D_OUT).transpose(1, 0, 2)  # [P, NK, D_OUT]
    return np.ascontiguousarray(w4.reshape(P, NK * D_OUT)).astype(ml_dtypes.float8_e4m3)


def make_in_maps(x: np.ndarray, weight: np.ndarray, w_scale: float | None = None):
    if w_scale is None:
        w_scale = _w_scale_like_reference(weight)
    x_flat = np.ascontiguousarray(
        x.reshape(TOK_TOTAL, D_IN).astype(np.float32, copy=False)
    )
    wb = _prep_weight(weight, w_scale)
    return [
        {"xs": x_flat[c * TOK : (c + 1) * TOK, :], "wb": wb} for c in range(N_CORES)
    ]


def run_on_hw(x: np.ndarray, weight: np.ndarray, trace: bool = False):
    """Compile + execute on the 8 NeuronCores.  Returns (y_full, results)."""
    from concourse.bass_utils import run_bass_kernel_spmd

    if trace:
        _install_ntff_hook()
    w_scale = _w_scale_like_reference(weight)
    nc = build_program(w_scale)
    in_maps = make_in_maps(x, weight, w_scale)
    res = run_bass_kernel_spmd(nc, in_maps, list(range(N_CORES)), trace=trace)
    y_full = np.concatenate(
        [np.asarray(res.results[c]["y"]).astype(np.float32) for c in range(N_CORES)],
        axis=0,
    ).reshape(x.shape[0], x.shape[1], D_OUT)
    return y_full, res


def _install_ntff_hook():
    """The agent image's antenv package lacks axon_hooks, so NTFF profiling
    silently degrades.  Recreate the hook module (ctypes into
    libaxon_pjrt.so) so run_bass_kernel_spmd(trace=True) works."""
    import types, ctypes, contextlib, os

    if "antenv.axon_hooks" in sys.modules:
        return
    so_path = "/opt/axon/libaxon_pjrt.so"
    if not os.path.exists(so_path):
        return
    lib = ctypes.CDLL(so_path)
    if not hasattr(lib, "axon_start_nrt_profile"):
        return
    lib.axon_start_nrt_profile.argtypes = [
        ctypes.POINTER(ctypes.c_int64),
        ctypes.c_size_t,
    ]
    lib.axon_start_nrt_profile.restype = ctypes.c_int64
    lib.axon_stop_nrt_profile.argtypes = [ctypes.c_char_p]
    lib.axon_stop_nrt_profile.restype = ctypes.c_int64

    @contextlib.contextmanager
    def _hook(output_dir, device_ids):
        import jax

        jax.devices()
        if device_ids:
            ids = (ctypes.c_int64 * len(device_ids))(*device_ids)
            rc = lib.axon_start_nrt_profile(ids, len(device_ids))
        else:
            rc = lib.axon_start_nrt_profile(None, 0)
        if rc != 0:
            raise RuntimeError(f"axon_start_nrt_profile rc={rc}")
        try:
            yield
        finally:
            n = lib.axon_stop_nrt_profile(str(output_dir).encode())
            print(f"profile: {n} file(s) written to {output_dir}", file=sys.stderr)

    mod = types.ModuleType("antenv.axon_hooks")
    mod.get_axon_ntff_profile_hook = lambda: _hook
    mod.set_axon_ntff_profile_hook = lambda h: None
    sys.modules["antenv.axon_hooks"] = mod

    import concourse.bass_utils as bu

    _orig_upload = bu.upload_artifacts

    def _safe_upload(tmpdir):
        try:
            return _orig_upload(tmpdir)
        except Exception as e:
            print(f"upload_artifacts skipped: {e}", file=sys.stderr)
            return tmpdir

    bu.upload_artifacts = _safe_upload


def kernel(x: np.ndarray, weight: np.ndarray) -> np.ndarray:
    y, _ = run_on_hw(x, weight, trace=False)
    return y


# revision 28
# speedup vs baseline: 1.0336x; 1.0336x over previous
"""BitLinear (activation int8-quant + ternary weight) Trainium2 kernel, v5.

Strategy (8 NeuronCores, token-parallel):
  - x [2,8192,2048] -> flat [16384, 2048]; core c gets a contiguous slice of
    2048 tokens.
  - Weight is pre-quantized on host (standard inference practice): w_scale =
    mean(|W|) via jax-CPU (bit-identical to reference), ternary w_q in
    {-1,0,+1} as bf16 (exact), transposed to [P, NK, D_OUT] k-tile layout.
  - Per-core activation path (all on device):
      abs-max per token -> qf = 127/s (dual-scalar op + reciprocal)
      ACT1: xq16 = f16(x*qf + 1536)  -- f16 ulp in [1024,2048) is exactly 1,
            so the downcast rounds RNE to integer.
      PE transpose (is_transpose matmul, 1 cyc/row) of each 128x128 f16 tile
            into PSUM -- no DMA-xbar transposes, which serialize against all
            in-flight DMA traffic.
      ACT2: xqT = bf16(psumT - 1536)  -- the un-magic pass doubles as the
            PSUM->SBUF evacuation, so the transpose costs only ~5% PE time.
  - PE: bf16 matmuls, stationary = xqT k-tile [128,128], moving = wq k-tile
    [128,512]; exact integer accumulation in fp32 PSUM (|acc| <= 2^18).
    GEMM runs in two 2-bank halves (n in {0,1}, then {2,3}) so 4 PSUM banks
    double-buffer the GEMM while 4 banks serve the transposes.
  - Post: ACT relu(acc*gf) -> A f32, DVE square -> bf16 out (0.2% norm err,
    gate is 2e-2).
  - Emission is software-pipelined: transposes for block m+1 are issued on
    the PE stream between GEMM m-1 and GEMM m, so ACT2(m+1) overlaps GEMM m.
"""

import sys

if "/opt/trn_rl_repo" not in sys.path:
    sys.path.insert(0, "/opt/trn_rl_repo")

import numpy as np

N_CORES = 8
P = 128
TOK_TOTAL = 16384
TOK = TOK_TOTAL // N_CORES  # 2048 tokens per core
D_IN = 2048
D_OUT = 2048
NK = D_IN // P  # 16 contraction tiles
NM = TOK // P  # 16 token blocks per core
NCHUNK = 512  # psum bank free dim (f32)
NN = D_OUT // NCHUNK  # 4
# f16 round-to-integer magic: adding 1536 puts v in [1408.5, 1663.5] inside
# [1024, 2048) where the f16 ulp is exactly 1, so the f32->f16 downcast
# rounds RNE to an integer.
MAGIC16 = 1536.0

_tile_patched = False


def _patch_tile_drain():
    """walrus in this container rejects >2 sem waits on the TileContext exit
    Drain ("Too many sync wait commands").  Split the excess waits onto
    explicit SP wait_ge instructions (same semantics: all waits complete
    before the semaphore free + final barrier)."""
    global _tile_patched
    if _tile_patched:
        return
    import concourse.tile as tile
    from bass_rust import ScopedClock

    def patched(self, tick_clock, wait_clock):
        nc_ = self.nc
        drain_inst = nc_.sync.drain()
        wait_clock.add_sem_waits(
            drain_inst.ins, ScopedClock({None: tick_clock.global_clock})
        )
        waits = list(drain_inst.ins.sync_info.on_wait or [])
        if len(waits) > 1:
            drain_inst.ins.sync_info.on_wait = waits[:1]
            name_to_sem = {}
            for key, h in self.sems.allocated().items():
                name_to_sem[getattr(h, "name", str(key))] = h
            for w in waits[1:]:
                nc_.sync.wait_ge(name_to_sem[w.ant_name], w.wait_value)
        nc_.all_engine_barrier()
        popped = nc_._tile_sem_poison_stack.pop()
        assert popped is self._sem_poison
        nc_.clear_and_free_semaphores(list(self.sems.allocated().values()))
        nc_.all_engine_barrier()

    tile.TileContext._drain_and_barrier = patched
    _tile_patched = True


def _split_excess_waits(nc, max_waits: int = 1):
    """walrus's setupSyncWait caps the number of semaphore waits a single
    instruction can carry.  Tile's scheduler freely attaches more.  Move the
    excess onto wait-only EventSemaphore carrier instructions inserted just
    before the over-subscribed instruction on the same engine (program order
    on one engine => identical semantics)."""
    from concourse import mybir

    n_split = 0
    for fn in nc.m.functions:
        for bb in fn.blocks:
            insts = bb.instructions
            i = 0
            while i < len(insts):
                inst = insts[i]
                si = getattr(inst, "sync_info", None)
                waits = list(si.on_wait) if (si is not None and si.on_wait) else []
                limit = 0 if type(inst).__name__ == "InstDmaTransposeAnt" else max_waits
                if len(waits) <= limit:
                    i += 1
                    continue
                keep = waits[-limit:] if limit else []
                extras = waits[: len(waits) - limit]
                pos = i
                for j in range(0, len(extras), max_waits):
                    ev = mybir.InstEventSemaphore(
                        name=f"wsplit_{inst.name}_{j}_{n_split}",
                        engine=inst.engine,
                        ins=[],
                        outs=[],
                        sync_info=mybir.SyncInfo(
                            on_wait=extras[j : j + max_waits], on_update=[]
                        ),
                    )
                    try:
                        nc.register_instruction(ev, overwrite=True)
                    except Exception:
                        pass
                    insts.insert(pos, ev)
                    pos += 1
                inst.sync_info.on_wait = keep
                n_split += 1
                i = pos + 1
    return n_split


def build_program(w_scale: float):
    """Build the per-core Bass program (same program runs SPMD on all 8
    cores; per-core data arrives via the input map)."""
    import concourse.bass as bass
    import concourse.tile as tile
    from concourse import mybir
    from concourse.masks import make_identity

    f32 = mybir.dt.float32
    f16 = mybir.dt.float16
    bf16 = mybir.dt.bfloat16
    fp8 = mybir.dt.float8e4
    AF = mybir.ActivationFunctionType
    ALU = mybir.AluOpType
    AX = mybir.AxisListType

    _patch_tile_drain()

    ws127 = float(np.float32(w_scale) / np.float32(127.0))

    nc = bass.Bass("TRN2", target_bir_lowering=False, debug=False)
    xs = nc.dram_tensor("xs", [TOK, D_IN], f32, kind="ExternalInput").ap()
    wb = nc.dram_tensor("wb", [P, NK * D_OUT], fp8, kind="ExternalInput").ap()
    y = nc.dram_tensor("y", [TOK, D_OUT], bf16, kind="ExternalOutput").ap()

    with tile.TileContext(nc) as tc:
        with (
            tc.tile_pool(name="wq", bufs=1) as wq_pool,
            tc.tile_pool(name="xin", bufs=3) as x_pool,
            tc.tile_pool(name="xq16", bufs=3) as xq_pool,
            tc.tile_pool(name="xqt", bufs=3) as xqt_pool,
            tc.tile_pool(name="scal", bufs=24) as s_pool,
            tc.tile_pool(name="tpsum", bufs=4, space="PSUM") as tpsum_pool,
            tc.tile_pool(name="gpsum", bufs=4, space="PSUM") as gpsum_pool,
            tc.tile_pool(name="outa", bufs=2) as a_pool,
            tc.tile_pool(name="outy", bufs=2) as y_pool,
            tc.tile_pool(name="consts", bufs=1) as c_pool,
        ):
            cmagic = c_pool.tile([P, 1], f32)
            nc.vector.memset(cmagic[:], MAGIC16)
            warm = c_pool.tile([P, 1], f32)
            nc.scalar.activation(warm[:], cmagic[:], AF.Identity, bias=0.0, scale=1.0)
            cneg = c_pool.tile([P, 1], f32)
            nc.vector.memset(cneg[:], -MAGIC16)
            ident = c_pool.tile([P, P], f16)
            make_identity(nc, ident[:])

            # persistent ternary W^T bf16 [128, k*D_OUT]; per-k chunk loads
            # split across the gpsimd and vector queues so the full 8.4MB
            # lands by ~15us
            wqd = wq_pool.tile([P, NK * D_OUT], fp8)
            wgate = c_pool.tile([P, 4], f32)

            state = {}

            def emit_quant(m):
                xf = x_pool.tile([P, D_IN], f32, tag="xf", name=f"xf_{m}")
                s0 = s_pool.tile([P, 1], f32, tag="s0", name=f"s0_{m}")
                if m == 0:
                    # split the first x load + absmax into halves so the
                    # critical path overlaps the DMA
                    H = D_IN // 2
                    nc.sync.dma_start(xf[:, 0:H], xs[0:P, 0:H])
                    nc.sync.dma_start(xf[:, H:], xs[0:P, H:])
                    # gate all W-chunk loads behind the first x half so
                    # x0 gets full HBM bandwidth on the critical path
                    nc.gpsimd.tensor_copy(wgate[:], xf[:, 0:4])
                    for k in range(NK):
                        nc.gpsimd.dma_start(
                            wqd[:, k * D_OUT : (k + 1) * D_OUT],
                            wb[:, k * D_OUT : (k + 1) * D_OUT],
                        )
                    sh = s_pool.tile([P, 1], f32, tag="sh", name="sh_0")
                    nc.vector.tensor_reduce(
                        sh[:], xf[:, 0:H], AX.X, ALU.max, apply_absolute_value=True
                    )
                    s1h = s_pool.tile([P, 1], f32, tag="s1h", name="s1h_0")
                    nc.vector.tensor_reduce(
                        s1h[:], xf[:, H:], AX.X, ALU.max, apply_absolute_value=True
                    )
                    nc.vector.tensor_tensor(s0[:], sh[:], s1h[:], ALU.max)
                else:
                    nc.sync.dma_start(xf[:], xs[m * P : (m + 1) * P, :])
                    nc.vector.tensor_reduce(
                        s0[:], xf[:], AX.X, ALU.max, apply_absolute_value=True
                    )
                u = s_pool.tile([P, 1], f32, tag="u", name=f"u_{m}")
                nc.vector.tensor_scalar(
                    u[:], s0[:], 1e-5, 1.0 / 127.0, ALU.max, ALU.mult
                )
                qf = s_pool.tile([P, 1], f32, tag="qf", name=f"qf_{m}")
                nc.vector.reciprocal(qf[:], u[:])
                gf = s_pool.tile([P, 1], f32, tag="gf", name=f"gf_{m}")
                nc.vector.tensor_scalar(gf[:], qf[:], ws127, None, ALU.mult)
                xq = xq_pool.tile([P, D_IN], f16, tag="xq", name=f"xq_{m}")
                nc.scalar.activation(
                    xq[:], xf[:], AF.Identity, bias=cmagic[:, 0:1], scale=qf[:, 0:1]
                )
                state[m] = {"xq": xq, "gf": gf}

            def emit_transpose(m):
                st = state[m]
                xq = st["xq"]
                xqt = xqt_pool.tile([P, NK * P], bf16, tag="xqt", name=f"xqt_{m}")
                for h in range(2):
                    tp = tpsum_pool.tile([P, 8 * P], f16, tag="tp", name=f"tp_{m}_{h}")
                    # one PSUM zero-region (2KB bank): start once, stop once,
                    # each transpose writes its own 256B slice exactly once
                    for j in range(8):
                        k = 8 * h + j
                        nc.tensor.matmul(
                            tp[:, j * P : (j + 1) * P],
                            xq[:, k * P : (k + 1) * P],
                            ident[:],
                            is_transpose=True,
                            start=(j == 0),
                            stop=(j == 7),
                            skip_group_check=True,
                        )
                    # un-magic doubles as PSUM->SBUF evacuation
                    nc.scalar.activation(
                        xqt[:, h * 8 * P : (h + 1) * 8 * P],
                        tp[:],
                        AF.Identity,
                        bias=cneg[:, 0:1],
                        scale=1.0,
                    )
                st["xqt"] = xqt

            def emit_gemm(m):
                st = state[m]
                xqt = st["xqt"]
                gf = st["gf"]
                A = a_pool.tile([P, D_OUT], f32, tag="A", name=f"A_{m}")
                Y = y_pool.tile([P, D_OUT], bf16, tag="Y", name=f"Y_{m}")
                last = m == NM - 1
                for half in range(2):
                    pss = []
                    for n in (2 * half, 2 * half + 1):
                        ps = gpsum_pool.tile(
                            [P, NCHUNK], f32, tag="ps", name=f"ps_{m}_{n}"
                        )
                        pss.append((n, ps))
                    for k in range(NK):
                        for n, ps in pss:
                            nc.tensor.matmul(
                                ps[:],
                                xqt[:, k * P : (k + 1) * P],
                                wqd[:, k * D_OUT + n * NCHUNK : k * D_OUT + (n + 1) * NCHUNK],
                                start=(k == 0),
                                stop=(k == NK - 1),
                            )
                    for n, ps in pss:
                        nc.scalar.activation(
                            A[:, n * NCHUNK : (n + 1) * NCHUNK],
                            ps[:],
                            AF.Relu,
                            bias=0.0,
                            scale=gf[:, 0:1],
                        )
                        if last:
                            ns = slice(n * NCHUNK, (n + 1) * NCHUNK)
                            nc.vector.tensor_tensor(
                                Y[:, ns], A[:, ns], A[:, ns], ALU.mult
                            )
                            nc.sync.dma_start(y[m * P : (m + 1) * P, ns], Y[:, ns])
                    if not last:
                        hs = slice(half * 2 * NCHUNK, (half + 1) * 2 * NCHUNK)
                        nc.vector.tensor_tensor(Y[:, hs], A[:, hs], A[:, hs], ALU.mult)
                        nc.sync.dma_start(y[m * P : (m + 1) * P, hs], Y[:, hs])

            # software pipeline on the PE stream:
            #   T0, G0, T1, T2, G1, T3, G2, ... (block-1 quant isn't ready
            #   when G0 starts, so T1 must not block the queue before G0)
            emit_quant(0)
            emit_transpose(0)
            emit_quant(1)
            emit_quant(2)
            emit_gemm(0)
            emit_transpose(1)
            for m in range(3, NM):
                emit_quant(m)
                emit_transpose(m - 1)
                emit_gemm(m - 2)
            emit_transpose(NM - 1)
            emit_gemm(NM - 2)
            emit_gemm(NM - 1)

    _split_excess_waits(nc)
    return nc


def _w_scale_like_reference(weight: np.ndarray) -> float:
    """mean(|W|) computed with jax on CPU so it is bit-identical to the
    reference's jnp.mean(jnp.abs(weight))."""
    try:
        import jax
        import jax.numpy as jnp

        cpu = jax.devices("cpu")[0]
        with jax.default_device(cpu):
            return float(jnp.mean(jnp.abs(jnp.asarray(weight, dtype=jnp.float32))))
    except Exception:
        return float(np.float32(np.abs(weight).astype(np.float64).mean()))


def _prep_weight(weight: np.ndarray, w_scale: float) -> np.ndarray:
    """Host-side weight quantization (offline in a real deployment): ternary
    {-1,0,1} with threshold 0.5*w_scale (same f32 compares as the reference),
    transposed, bf16, k-tile layout wb[p, k*D_OUT + o] = W_q^T[128k+p, o]."""
    import ml_dtypes

    w = weight.astype(np.float32, copy=False)
    thr = np.float32(0.5) * np.float32(w_scale)
    wq = np.where(
        w > thr, np.float32(1.0), np.where(w < -thr, np.float32(-1.0), np.float32(0.0))
    )
    wqT = np.ascontiguousarray(wq.T)  # [in, out]
    w4 = wqT.reshape(NK, P, D_OUT).transpose(1, 0, 2)  # [P, NK, D_OUT]
    return np.ascontiguousarray(w4.reshape(P, NK * D_OUT)).astype(ml_dtypes.float8_e4m3)


def make_in_maps(x: np.ndarray, weight: np.ndarray, w_scale: float | None = None):
    if w_scale is None:
        w_scale = _w_scale_like_reference(weight)
    x_flat = np.ascontiguousarray(
        x.reshape(TOK_TOTAL, D_IN).astype(np.float32, copy=False)
    )
    wb = _prep_weight(weight, w_scale)
    return [
        {"xs": x_flat[c * TOK : (c + 1) * TOK, :], "wb": wb} for c in range(N_CORES)
    ]


def run_on_hw(x: np.ndarray, weight: np.ndarray, trace: bool = False):
    """Compile + execute on the 8 NeuronCores.  Returns (y_full, results)."""
    from concourse.bass_utils import run_bass_kernel_spmd

    if trace:
        _install_ntff_hook()
    w_scale = _w_scale_like_reference(weight)
    nc = build_program(w_scale)
    in_maps = make_in_maps(x, weight, w_scale)
    res = run_bass_kernel_spmd(nc, in_maps, list(range(N_CORES)), trace=trace)
    y_full = np.concatenate(
        [np.asarray(res.results[c]["y"]).astype(np.float32) for c in range(N_CORES)],
        axis=0,
    ).reshape(x.shape[0], x.shape[1], D_OUT)
    return y_full, res


def _install_ntff_hook():
    """The agent image's antenv package lacks axon_hooks, so NTFF profiling
    silently degrades.  Recreate the hook module (ctypes into
    libaxon_pjrt.so) so run_bass_kernel_spmd(trace=True) works."""
    import types, ctypes, contextlib, os

    if "antenv.axon_hooks" in sys.modules:
        return
    so_path = "/opt/axon/libaxon_pjrt.so"
    if not os.path.exists(so_path):
        return
    lib = ctypes.CDLL(so_path)
    if not hasattr(lib, "axon_start_nrt_profile"):
        return
    lib.axon_start_nrt_profile.argtypes = [
        ctypes.POINTER(ctypes.c_int64),
        ctypes.c_size_t,
    ]
    lib.axon_start_nrt_profile.restype = ctypes.c_int64
    lib.axon_stop_nrt_profile.argtypes = [ctypes.c_char_p]
    lib.axon_stop_nrt_profile.restype = ctypes.c_int64

    @contextlib.contextmanager
    def _hook(output_dir, device_ids):
        import jax

        jax.devices()
        if device_ids:
            ids = (ctypes.c_int64 * len(device_ids))(*device_ids)
            rc = lib.axon_start_nrt_profile(ids, len(device_ids))
        else:
            rc = lib.axon_start_nrt_profile(None, 0)
        if rc != 0:
            raise RuntimeError(f"axon_start_nrt_profile rc={rc}")
        try:
            yield
        finally:
            n = lib.axon_stop_nrt_profile(str(output_dir).encode())
            print(f"profile: {n} file(s) written to {output_dir}", file=sys.stderr)

    mod = types.ModuleType("antenv.axon_hooks")
    mod.get_axon_ntff_profile_hook = lambda: _hook
    mod.set_axon_ntff_profile_hook = lambda h: None
    sys.modules["antenv.axon_hooks"] = mod

    import concourse.bass_utils as bu

    _orig_upload = bu.upload_artifacts

    def _safe_upload(tmpdir):
        try:
            return _orig_upload(tmpdir)
        except Exception as e:
            print(f"upload_artifacts skipped: {e}", file=sys.stderr)
            return tmpdir

    bu.upload_artifacts = _safe_upload


def kernel(x: np.ndarray, weight: np.ndarray) -> np.ndarray:
    y, _ = run_on_hw(x, weight, trace=False)
    return y


# revision 29
# speedup vs baseline: 1.0722x; 1.0373x over previous
"""BitLinear (activation int8-quant + ternary weight) Trainium2 kernel, v5.

Strategy (8 NeuronCores, token-parallel):
  - x [2,8192,2048] -> flat [16384, 2048]; core c gets a contiguous slice of
    2048 tokens.
  - Weight is pre-quantized on host (standard inference practice): w_scale =
    mean(|W|) via jax-CPU (bit-identical to reference), ternary w_q in
    {-1,0,+1} as bf16 (exact), transposed to [P, NK, D_OUT] k-tile layout.
  - Per-core activation path (all on device):
      abs-max per token -> qf = 127/s (dual-scalar op + reciprocal)
      ACT1: xq16 = f16(x*qf + 1536)  -- f16 ulp in [1024,2048) is exactly 1,
            so the downcast rounds RNE to integer.
      PE transpose (is_transpose matmul, 1 cyc/row) of each 128x128 f16 tile
            into PSUM -- no DMA-xbar transposes, which serialize against all
            in-flight DMA traffic.
      ACT2: xqT = bf16(psumT - 1536)  -- the un-magic pass doubles as the
            PSUM->SBUF evacuation, so the transpose costs only ~5% PE time.
  - PE: bf16 matmuls, stationary = xqT k-tile [128,128], moving = wq k-tile
    [128,512]; exact integer accumulation in fp32 PSUM (|acc| <= 2^18).
    GEMM runs in two 2-bank halves (n in {0,1}, then {2,3}) so 4 PSUM banks
    double-buffer the GEMM while 4 banks serve the transposes.
  - Post: ACT relu(acc*gf) -> A f32, DVE square -> bf16 out (0.2% norm err,
    gate is 2e-2).
  - Emission is software-pipelined: transposes for block m+1 are issued on
    the PE stream between GEMM m-1 and GEMM m, so ACT2(m+1) overlaps GEMM m.
"""

import sys

if "/opt/trn_rl_repo" not in sys.path:
    sys.path.insert(0, "/opt/trn_rl_repo")

import numpy as np

N_CORES = 8
P = 128
TOK_TOTAL = 16384
TOK = TOK_TOTAL // N_CORES  # 2048 tokens per core
D_IN = 2048
D_OUT = 2048
NK = D_IN // P  # 16 contraction tiles
NM = TOK // P  # 16 token blocks per core
NCHUNK = 512  # psum bank free dim (f32)
NN = D_OUT // NCHUNK  # 4
# f16 round-to-integer magic: adding 1536 puts v in [1408.5, 1663.5] inside
# [1024, 2048) where the f16 ulp is exactly 1, so the f32->f16 downcast
# rounds RNE to an integer.
MAGIC16 = 1536.0

_tile_patched = False


def _patch_tile_drain():
    """walrus in this container rejects >2 sem waits on the TileContext exit
    Drain ("Too many sync wait commands").  Split the excess waits onto
    explicit SP wait_ge instructions (same semantics: all waits complete
    before the semaphore free + final barrier)."""
    global _tile_patched
    if _tile_patched:
        return
    import concourse.tile as tile
    from bass_rust import ScopedClock

    def patched(self, tick_clock, wait_clock):
        nc_ = self.nc
        drain_inst = nc_.sync.drain()
        wait_clock.add_sem_waits(
            drain_inst.ins, ScopedClock({None: tick_clock.global_clock})
        )
        waits = list(drain_inst.ins.sync_info.on_wait or [])
        if len(waits) > 1:
            drain_inst.ins.sync_info.on_wait = waits[:1]
            name_to_sem = {}
            for key, h in self.sems.allocated().items():
                name_to_sem[getattr(h, "name", str(key))] = h
            for w in waits[1:]:
                nc_.sync.wait_ge(name_to_sem[w.ant_name], w.wait_value)
        nc_.all_engine_barrier()
        popped = nc_._tile_sem_poison_stack.pop()
        assert popped is self._sem_poison
        nc_.clear_and_free_semaphores(list(self.sems.allocated().values()))
        nc_.all_engine_barrier()

    tile.TileContext._drain_and_barrier = patched
    _tile_patched = True


def _split_excess_waits(nc, max_waits: int = 1):
    """walrus's setupSyncWait caps the number of semaphore waits a single
    instruction can carry.  Tile's scheduler freely attaches more.  Move the
    excess onto wait-only EventSemaphore carrier instructions inserted just
    before the over-subscribed instruction on the same engine (program order
    on one engine => identical semantics)."""
    from concourse import mybir

    n_split = 0
    for fn in nc.m.functions:
        for bb in fn.blocks:
            insts = bb.instructions
            i = 0
            while i < len(insts):
                inst = insts[i]
                si = getattr(inst, "sync_info", None)
                waits = list(si.on_wait) if (si is not None and si.on_wait) else []
                limit = 0 if type(inst).__name__ == "InstDmaTransposeAnt" else max_waits
                if len(waits) <= limit:
                    i += 1
                    continue
                keep = waits[-limit:] if limit else []
                extras = waits[: len(waits) - limit]
                pos = i
                for j in range(0, len(extras), max_waits):
                    ev = mybir.InstEventSemaphore(
                        name=f"wsplit_{inst.name}_{j}_{n_split}",
                        engine=inst.engine,
                        ins=[],
                        outs=[],
                        sync_info=mybir.SyncInfo(
                            on_wait=extras[j : j + max_waits], on_update=[]
                        ),
                    )
                    try:
                        nc.register_instruction(ev, overwrite=True)
                    except Exception:
                        pass
                    insts.insert(pos, ev)
                    pos += 1
                inst.sync_info.on_wait = keep
                n_split += 1
                i = pos + 1
    return n_split


def build_program(w_scale: float):
    """Build the per-core Bass program (same program runs SPMD on all 8
    cores; per-core data arrives via the input map)."""
    import concourse.bass as bass
    import concourse.tile as tile
    from concourse import mybir
    from concourse.masks import make_identity

    f32 = mybir.dt.float32
    f16 = mybir.dt.float16
    bf16 = mybir.dt.bfloat16
    fp8 = mybir.dt.float8e4
    AF = mybir.ActivationFunctionType
    ALU = mybir.AluOpType
    AX = mybir.AxisListType

    _patch_tile_drain()

    ws127 = float(np.float32(w_scale) / np.float32(127.0))

    nc = bass.Bass("TRN2", target_bir_lowering=False, debug=False)
    xs = nc.dram_tensor("xs", [TOK, D_IN], f32, kind="ExternalInput").ap()
    wb = nc.dram_tensor("wb", [P, NK * D_OUT], fp8, kind="ExternalInput").ap()
    y = nc.dram_tensor("y", [TOK, D_OUT], bf16, kind="ExternalOutput").ap()

    with tile.TileContext(nc) as tc:
        with (
            tc.tile_pool(name="wq", bufs=1) as wq_pool,
            tc.tile_pool(name="xin", bufs=3) as x_pool,
            tc.tile_pool(name="xq16", bufs=3) as xq_pool,
            tc.tile_pool(name="xqt", bufs=3) as xqt_pool,
            tc.tile_pool(name="scal", bufs=24) as s_pool,
            tc.tile_pool(name="tpsum", bufs=4, space="PSUM") as tpsum_pool,
            tc.tile_pool(name="gpsum", bufs=4, space="PSUM") as gpsum_pool,
            tc.tile_pool(name="outa", bufs=2) as a_pool,
            tc.tile_pool(name="outy", bufs=2) as y_pool,
            tc.tile_pool(name="consts", bufs=1) as c_pool,
        ):
            cmagic = c_pool.tile([P, 1], f32)
            nc.vector.memset(cmagic[:], MAGIC16)
            warm = c_pool.tile([P, 1], f32)
            nc.scalar.activation(warm[:], cmagic[:], AF.Identity, bias=0.0, scale=1.0)
            cneg = c_pool.tile([P, 1], f32)
            nc.vector.memset(cneg[:], -MAGIC16)
            ident = c_pool.tile([P, P], f16)
            make_identity(nc, ident[:])

            # persistent ternary W^T bf16 [128, k*D_OUT]; per-k chunk loads
            # split across the gpsimd and vector queues so the full 8.4MB
            # lands by ~15us
            wqd = wq_pool.tile([P, NK * D_OUT], fp8)
            for k in range(0, NK, 2):
                nc.gpsimd.dma_start(
                    wqd[:, k * D_OUT : (k + 1) * D_OUT],
                    wb[:, k * D_OUT : (k + 1) * D_OUT],
                )

            state = {}

            def emit_quant(m):
                xf = x_pool.tile([P, D_IN], f32, tag="xf", name=f"xf_{m}")
                s0 = s_pool.tile([P, 1], f32, tag="s0", name=f"s0_{m}")
                if m == 0:
                    # split the first x load + absmax into halves so the
                    # critical path overlaps the DMA
                    H = D_IN // 2
                    nc.sync.dma_start(xf[:, 0:H], xs[0:P, 0:H])
                    nc.sync.dma_start(xf[:, H:], xs[0:P, H:])
                    for k in (1, 3, 5, 7):
                        nc.sync.dma_start(
                            wqd[:, k * D_OUT : (k + 1) * D_OUT],
                            wb[:, k * D_OUT : (k + 1) * D_OUT],
                        )
                    sh = s_pool.tile([P, 1], f32, tag="sh", name="sh_0")
                    nc.vector.tensor_reduce(
                        sh[:], xf[:, 0:H], AX.X, ALU.max, apply_absolute_value=True
                    )
                    s1h = s_pool.tile([P, 1], f32, tag="s1h", name="s1h_0")
                    nc.vector.tensor_reduce(
                        s1h[:], xf[:, H:], AX.X, ALU.max, apply_absolute_value=True
                    )
                    nc.vector.tensor_tensor(s0[:], sh[:], s1h[:], ALU.max)
                else:
                    nc.sync.dma_start(xf[:], xs[m * P : (m + 1) * P, :])
                    if m == 1:
                        for k in (9, 11, 13, 15):
                            nc.sync.dma_start(
                                wqd[:, k * D_OUT : (k + 1) * D_OUT],
                                wb[:, k * D_OUT : (k + 1) * D_OUT],
                            )
                    nc.vector.tensor_reduce(
                        s0[:], xf[:], AX.X, ALU.max, apply_absolute_value=True
                    )
                u = s_pool.tile([P, 1], f32, tag="u", name=f"u_{m}")
                nc.vector.tensor_scalar(
                    u[:], s0[:], 1e-5, 1.0 / 127.0, ALU.max, ALU.mult
                )
                qf = s_pool.tile([P, 1], f32, tag="qf", name=f"qf_{m}")
                nc.vector.reciprocal(qf[:], u[:])
                gf = s_pool.tile([P, 1], f32, tag="gf", name=f"gf_{m}")
                nc.vector.tensor_scalar(gf[:], qf[:], ws127, None, ALU.mult)
                xq = xq_pool.tile([P, D_IN], f16, tag="xq", name=f"xq_{m}")
                nc.scalar.activation(
                    xq[:], xf[:], AF.Identity, bias=cmagic[:, 0:1], scale=qf[:, 0:1]
                )
                state[m] = {"xq": xq, "gf": gf}

            def emit_transpose(m):
                st = state[m]
                xq = st["xq"]
                xqt = xqt_pool.tile([P, NK * P], bf16, tag="xqt", name=f"xqt_{m}")
                for h in range(2):
                    tp = tpsum_pool.tile([P, 8 * P], f16, tag="tp", name=f"tp_{m}_{h}")
                    # one PSUM zero-region (2KB bank): start once, stop once,
                    # each transpose writes its own 256B slice exactly once
                    for j in range(8):
                        k = 8 * h + j
                        nc.tensor.matmul(
                            tp[:, j * P : (j + 1) * P],
                            xq[:, k * P : (k + 1) * P],
                            ident[:],
                            is_transpose=True,
                            start=(j == 0),
                            stop=(j == 7),
                            skip_group_check=True,
                        )
                    # un-magic doubles as PSUM->SBUF evacuation
                    nc.scalar.activation(
                        xqt[:, h * 8 * P : (h + 1) * 8 * P],
                        tp[:],
                        AF.Identity,
                        bias=cneg[:, 0:1],
                        scale=1.0,
                    )
                st["xqt"] = xqt

            def emit_gemm(m):
                st = state[m]
                xqt = st["xqt"]
                gf = st["gf"]
                A = a_pool.tile([P, D_OUT], f32, tag="A", name=f"A_{m}")
                Y = y_pool.tile([P, D_OUT], bf16, tag="Y", name=f"Y_{m}")
                for half in range(2):
                    pss = []
                    for n in (2 * half, 2 * half + 1):
                        ps = gpsum_pool.tile(
                            [P, NCHUNK], f32, tag="ps", name=f"ps_{m}_{n}"
                        )
                        pss.append((n, ps))
                    for k in range(NK):
                        for n, ps in pss:
                            nc.tensor.matmul(
                                ps[:],
                                xqt[:, k * P : (k + 1) * P],
                                wqd[:, k * D_OUT + n * NCHUNK : k * D_OUT + (n + 1) * NCHUNK],
                                start=(k == 0),
                                stop=(k == NK - 1),
                            )
                    for n, ps in pss:
                        nc.scalar.activation(
                            A[:, n * NCHUNK : (n + 1) * NCHUNK],
                            ps[:],
                            AF.Relu,
                            bias=0.0,
                            scale=gf[:, 0:1],
                        )
                    hs = slice(half * 2 * NCHUNK, (half + 1) * 2 * NCHUNK)
                    nc.vector.tensor_tensor(Y[:, hs], A[:, hs], A[:, hs], ALU.mult)
                    nc.sync.dma_start(y[m * P : (m + 1) * P, hs], Y[:, hs])

            # software pipeline on the PE stream:
            #   T0, G0, T1, T2, G1, T3, G2, ... (block-1 quant isn't ready
            #   when G0 starts, so T1 must not block the queue before G0)
            emit_quant(0)
            emit_transpose(0)
            emit_quant(1)
            emit_quant(2)
            emit_gemm(0)
            emit_transpose(1)
            for m in range(3, NM):
                emit_quant(m)
                emit_transpose(m - 1)
                emit_gemm(m - 2)
            emit_transpose(NM - 1)
            emit_gemm(NM - 2)
            emit_gemm(NM - 1)

    _split_excess_waits(nc)
    return nc


def _w_scale_like_reference(weight: np.ndarray) -> float:
    """mean(|W|) computed with jax on CPU so it is bit-identical to the
    reference's jnp.mean(jnp.abs(weight))."""
    try:
        import jax
        import jax.numpy as jnp

        cpu = jax.devices("cpu")[0]
        with jax.default_device(cpu):
            return float(jnp.mean(jnp.abs(jnp.asarray(weight, dtype=jnp.float32))))
    except Exception:
        return float(np.float32(np.abs(weight).astype(np.float64).mean()))


def _prep_weight(weight: np.ndarray, w_scale: float) -> np.ndarray:
    """Host-side weight quantization (offline in a real deployment): ternary
    {-1,0,1} with threshold 0.5*w_scale (same f32 compares as the reference),
    transposed, bf16, k-tile layout wb[p, k*D_OUT + o] = W_q^T[128k+p, o]."""
    import ml_dtypes

    w = weight.astype(np.float32, copy=False)
    thr = np.float32(0.5) * np.float32(w_scale)
    wq = np.where(
        w > thr, np.float32(1.0), np.where(w < -thr, np.float32(-1.0), np.float32(0.0))
    )
    wqT = np.ascontiguousarray(wq.T)  # [in, out]
    w4 = wqT.reshape(NK, P, D_OUT).transpose(1, 0, 2)  # [P, NK, D_OUT]
    return np.ascontiguousarray(w4.reshape(P, NK * D_OUT)).astype(ml_dtypes.float8_e4m3)


def make_in_maps(x: np.ndarray, weight: np.ndarray, w_scale: float | None = None):
    if w_scale is None:
        w_scale = _w_scale_like_reference(weight)
    x_flat = np.ascontiguousarray(
        x.reshape(TOK_TOTAL, D_IN).astype(np.float32, copy=False)
    )
    wb = _prep_weight(weight, w_scale)
    return [
        {"xs": x_flat[c * TOK : (c + 1) * TOK, :], "wb": wb} for c in range(N_CORES)
    ]


def run_on_hw(x: np.ndarray, weight: np.ndarray, trace: bool = False):
    """Compile + execute on the 8 NeuronCores.  Returns (y_full, results)."""
    from concourse.bass_utils import run_bass_kernel_spmd

    if trace:
        _install_ntff_hook()
    w_scale = _w_scale_like_reference(weight)
    nc = build_program(w_scale)
    in_maps = make_in_maps(x, weight, w_scale)
    res = run_bass_kernel_spmd(nc, in_maps, list(range(N_CORES)), trace=trace)
    y_full = np.concatenate(
        [np.asarray(res.results[c]["y"]).astype(np.float32) for c in range(N_CORES)],
        axis=0,
    ).reshape(x.shape[0], x.shape[1], D_OUT)
    return y_full, res


def _install_ntff_hook():
    """The agent image's antenv package lacks axon_hooks, so NTFF profiling
    silently degrades.  Recreate the hook module (ctypes into
    libaxon_pjrt.so) so run_bass_kernel_spmd(trace=True) works."""
    import types, ctypes, contextlib, os

    if "antenv.axon_hooks" in sys.modules:
        return
    so_path = "/opt/axon/libaxon_pjrt.so"
    if not os.path.exists(so_path):
        return
    lib = ctypes.CDLL(so_path)
    if not hasattr(lib, "axon_start_nrt_profile"):
        return
    lib.axon_start_nrt_profile.argtypes = [
        ctypes.POINTER(ctypes.c_int64),
        ctypes.c_size_t,
    ]
    lib.axon_start_nrt_profile.restype = ctypes.c_int64
    lib.axon_stop_nrt_profile.argtypes = [ctypes.c_char_p]
    lib.axon_stop_nrt_profile.restype = ctypes.c_int64

    @contextlib.contextmanager
    def _hook(output_dir, device_ids):
        import jax

        jax.devices()
        if device_ids:
            ids = (ctypes.c_int64 * len(device_ids))(*device_ids)
            rc = lib.axon_start_nrt_profile(ids, len(device_ids))
        else:
            rc = lib.axon_start_nrt_profile(None, 0)
        if rc != 0:
            raise RuntimeError(f"axon_start_nrt_profile rc={rc}")
        try:
            yield
        finally:
            n = lib.axon_stop_nrt_profile(str(output_dir).encode())
            print(f"profile: {n} file(s) written to {output_dir}", file=sys.stderr)

    mod = types.ModuleType("antenv.axon_hooks")
    mod.get_axon_ntff_profile_hook = lambda: _hook
    mod.set_axon_ntff_profile_hook = lambda h: None
    sys.modules["antenv.axon_hooks"] = mod

    import concourse.bass_utils as bu

    _orig_upload = bu.upload_artifacts

    def _safe_upload(tmpdir):
        try:
            return _orig_upload(tmpdir)
        except Exception as e:
            print(f"upload_artifacts skipped: {e}", file=sys.stderr)
            return tmpdir

    bu.upload_artifacts = _safe_upload


def kernel(x: np.ndarray, weight: np.ndarray) -> np.ndarray:
    y, _ = run_on_hw(x, weight, trace=False)
    return y
